# revision 7
# baseline (speedup 1.0000x reference)
"""nn_Block_498216206781: GQA causal attention + top-2-of-32 MoE block on 8
Trainium2 NeuronCores.

Strategy:
  Launch A (token-sharded, SPMD over 8 cores): pre-norm GQA causal attention
    + residual + pre-norm router logits. All matmuls fp32 (router top-2
    selection is sensitive: min top2-vs-3rd prob gap ~1e-6).
  Host glue: softmax/top-2 routing, capacity-padded per-expert token gather
    (all-to-all dispatch done host-side), aux loss.
  Launch B (expert-parallel, 4 experts/core): bf16 expert FFNs
    cw * gelu_tanh(x @ W1) @ W2 over gathered tokens only (~16x FLOP saving
    vs dense reference scan).
  Host: scatter-add expert outputs, final residual.
"""
from contextlib import ExitStack

import numpy as np
import ml_dtypes

import concourse.bass as bass
import concourse.tile as tile
from concourse import bacc, mybir
from concourse import masks
from concourse.bass_utils import run_bass_kernel_spmd

dt = mybir.dt
F32 = dt.float32
BF16 = dt.bfloat16
AF = mybir.ActivationFunctionType
OP = mybir.AluOpType

# problem shapes (hardcoded per contract)
B, S, D = 2, 1024, 768
H, HK, DH = 12, 2, 64
E, F = 32, 3072
TOP_K = 2
EPS = 1e-6
N_CORES = 8

T = 256               # query rows per core (B*S / N_CORES)
KD = D // 128         # 6
QT_TILES = T // 128   # 2
ST_TILES = S // 128   # 8
EPC = E // N_CORES    # 4 experts per core
KF = F // 128         # 24

# Head slot permutation: slot j holds original head HEAD_PERM[j], chosen so
# slot parity == kv-head index (matmul operands need equal base partitions).
HEAD_PERM = [0, 6, 1, 7, 2, 8, 3, 9, 4, 10, 5, 11]

_CACHE = {}
_HOST_CACHE = {}
LAST_EXEC_NS = {}
PROFILE = False


# ---------------------------------------------------------------- kernel A --
def _build_a():
    nc = bacc.Bacc("TRN2", target_bir_lowering=False, debug=False,
                   num_devices=N_CORES)
    io = {}
    for name, shape in [("x_own", [T, D]), ("x_full", [S, D]),
                        ("xT_full", [D, S]), ("maskT", [S, T]),
                        ("Wq", [D, D]), ("Wk", [D, HK * DH]), ("Wv", [D, HK * DH]),
                        ("Wo", [D, D]), ("Rw", [D, 32])]:
        io[name] = nc.dram_tensor(name, shape, F32, kind="ExternalInput")
    for name, shape in [("x_att", [T, D]), ("h2", [T, D]), ("logitsT", [32, T])]:
        io[name] = nc.dram_tensor(name, shape, F32, kind="ExternalOutput")
    with tile.TileContext(nc) as tc:
        with ExitStack() as ctx:
            _body_a(ctx, tc, io)
    nc.compile()
    return nc


def _body_a(ctx, tc, io):
    nc = tc.nc
    persist = ctx.enter_context(tc.tile_pool(name="persist", bufs=1))
    small = ctx.enter_context(tc.tile_pool(name="small", bufs=1))

    ident = persist.tile([128, 128], F32, name="ident", tag="ident")
    masks.make_identity(nc, ident[:])
    eps_t = persist.tile([128, 1], F32, name="eps_t", tag="eps_t")
    nc.vector.memset(eps_t[:], EPS)

    # ---- load inputs ----
    x_own = [persist.tile([128, D], F32, name=f"x_own{i}", tag=f"x_own{i}")
             for i in range(QT_TILES)]
    for i in range(QT_TILES):
        nc.sync.dma_start(x_own[i][:], io["x_own"].ap()[i * 128:(i + 1) * 128, :])
    x_full = [persist.tile([128, D], F32, name=f"x_full{i}", tag=f"x_full{i}")
              for i in range(ST_TILES)]
    for i in range(ST_TILES):
        nc.sync.dma_start(x_full[i][:], io["x_full"].ap()[i * 128:(i + 1) * 128, :])
    xT = [persist.tile([128, S], F32, name=f"xT{i}", tag=f"xT{i}")
          for i in range(KD)]
    for i in range(KD):
        nc.sync.dma_start(xT[i][:], io["xT_full"].ap()[i * 128:(i + 1) * 128, :])
    maskT = [persist.tile([128, T], F32, name=f"maskT{i}", tag=f"maskT{i}")
             for i in range(ST_TILES)]
    for i in range(ST_TILES):
        nc.gpsimd.dma_start(maskT[i][:], io["maskT"].ap()[i * 128:(i + 1) * 128, :])
    Wq = [persist.tile([128, D], F32, name=f"Wq{i}", tag=f"Wq{i}") for i in range(KD)]
    Wk = [persist.tile([128, HK * DH], F32, name=f"Wk{i}", tag=f"Wk{i}") for i in range(KD)]
    Wv = [persist.tile([128, HK * DH], F32, name=f"Wv{i}", tag=f"Wv{i}") for i in range(KD)]
    Wo = [persist.tile([128, D], F32, name=f"Wo{i}", tag=f"Wo{i}") for i in range(KD)]
    Rw = [persist.tile([128, 32], F32, name=f"Rw{i}", tag=f"Rw{i}") for i in range(KD)]
    for i in range(KD):
        nc.gpsimd.dma_start(Wq[i][:], io["Wq"].ap()[i * 128:(i + 1) * 128, :])
        nc.sync.dma_start(Wk[i][:], io["Wk"].ap()[i * 128:(i + 1) * 128, :])
        nc.sync.dma_start(Wv[i][:], io["Wv"].ap()[i * 128:(i + 1) * 128, :])
        nc.gpsimd.dma_start(Wo[i][:], io["Wo"].ap()[i * 128:(i + 1) * 128, :])
        nc.gpsimd.dma_start(Rw[i][:], io["Rw"].ap()[i * 128:(i + 1) * 128, :])

    # ---- per-token rms scales (1/sqrt(mean(x^2)+eps), Newton-refined) ----
    def rms_scales(tiles, n, tag):
        s = []
        for i in range(n):
            scr = small.tile([128, D], F32, name="rms_scr", tag="rms_scr")
            acc = small.tile([128, 1], F32, name=f"{tag}a{i}", tag=f"{tag}a{i}")
            nc.scalar.activation(scr[:], tiles[i][:], AF.Square, accum_out=acc[:])
            rt = small.tile([128, 1], F32, name=f"{tag}r{i}", tag=f"{tag}r{i}")
            nc.scalar.activation(rt[:], acc[:], AF.Sqrt, bias=eps_t[:], scale=1.0 / D)
            s0 = small.tile([128, 1], F32, name=f"{tag}s0{i}", tag=f"{tag}s0{i}")
            nc.vector.reciprocal(s0[:], rt[:])
            # Newton on rsqrt: ACT Sqrt LUT alone is only ~5e-6 accurate,
            # which would pollute router logits.
            m_t = small.tile([128, 1], F32, name=f"{tag}m{i}", tag=f"{tag}m{i}")
            nc.scalar.activation(m_t[:], acc[:], AF.Identity, bias=eps_t[:], scale=1.0 / D)
            t1 = small.tile([128, 1], F32, name=f"{tag}t1{i}", tag=f"{tag}t1{i}")
            nc.vector.tensor_tensor(out=t1[:], in0=s0[:], in1=s0[:], op=OP.mult)
            nc.vector.tensor_tensor(out=t1[:], in0=t1[:], in1=m_t[:], op=OP.mult)
            nc.vector.tensor_scalar(out=t1[:], in0=t1[:], scalar1=-0.5, scalar2=1.5,
                                    op0=OP.mult, op1=OP.add)
            si = small.tile([128, 1], F32, name=f"{tag}s{i}", tag=f"{tag}s{i}")
            nc.vector.tensor_tensor(out=si[:], in0=s0[:], in1=t1[:], op=OP.mult)
            s.append(si)
        return s

    s_own = rms_scales(x_own, QT_TILES, "so")
    s_full = rms_scales(x_full, ST_TILES, "sf")

    # ---- h_own token-major -> transpose -> hT_own [768, 256] ----
    h_own = [persist.tile([128, D], F32, name=f"h_own{i}", tag=f"h_own{i}")
             for i in range(QT_TILES)]
    for i in range(QT_TILES):
        nc.vector.tensor_scalar_mul(h_own[i][:], x_own[i][:], s_own[i][:])

    hT = [persist.tile([128, T], F32, name=f"hT{j}", tag=f"hT{j}") for j in range(KD)]
    with tc.tile_pool(name="tp_ps", bufs=2, space="PSUM") as tp_ps:
        for j in range(KD):
            for i in range(QT_TILES):
                pt = tp_ps.tile([128, 128], F32, name="tp", tag="tp")
                nc.tensor.transpose(pt[:], h_own[i][:, j * 128:(j + 1) * 128], ident[:])
                nc.scalar.copy(hT[j][:, i * 128:(i + 1) * 128], pt[:])

    # ---- QT [768, 256] (Wq carries the 1/8 score scale) ----
    QTt = [persist.tile([128, T], F32, name=f"QT{j}", tag=f"QT{j}") for j in range(KD)]
    with tc.tile_pool(name="qt_ps", bufs=2, space="PSUM") as qt_ps:
        for j in range(KD):
            pq = qt_ps.tile([128, T], F32, name="pq", tag="pq")
            for ki in range(KD):
                nc.tensor.matmul(pq[:], Wq[ki][:, j * 128:(j + 1) * 128], hT[ki][:],
                                 start=(ki == 0), stop=(ki == KD - 1))
            nc.scalar.copy(QTt[j][:], pq[:])

    # ---- KT, VT [128, 1024] feature-major ----
    KT = persist.tile([128, S], F32, name="KT", tag="KT")
    VT = persist.tile([128, S], F32, name="VT", tag="VT")
    with tc.tile_pool(name="kv_ps", bufs=2, space="PSUM") as kv_ps:
        for nj in range(2):
            sl = slice(nj * 512, (nj + 1) * 512)
            pk = kv_ps.tile([128, 512], F32, name="pk", tag="pk")
            for ki in range(KD):
                nc.tensor.matmul(pk[:], Wk[ki][:], xT[ki][:, sl],
                                 start=(ki == 0), stop=(ki == KD - 1))
            nc.scalar.copy(KT[:, sl], pk[:])
            pv = kv_ps.tile([128, 512], F32, name="pv", tag="pv")
            for ki in range(KD):
                nc.tensor.matmul(pv[:], Wv[ki][:], xT[ki][:, sl],
                                 start=(ki == 0), stop=(ki == KD - 1))
            nc.scalar.copy(VT[:, sl], pv[:])

    # ---- V_ext [1024, 130] token-major: [s*V_kv0 | 1 | s*V_kv1 | 1] ----
    # The appended ones-column turns the O matmul into O|rowsum, giving the
    # softmax denominator for free in token-major layout.
    V_ext = [persist.tile([128, 2 * DH + 2], F32, name=f"Vx{k}", tag=f"Vx{k}")
             for k in range(ST_TILES)]
    with tc.tile_pool(name="vt_ps", bufs=2, space="PSUM") as vt_ps:
        for kt in range(ST_TILES):
            pt = vt_ps.tile([128, 128], F32, name="vtp", tag="vtp")
            nc.tensor.transpose(pt[:], VT[:, kt * 128:(kt + 1) * 128], ident[:])
            nc.vector.tensor_scalar_mul(V_ext[kt][:, 0:DH], pt[:, 0:DH], s_full[kt][:])
            nc.vector.tensor_scalar_mul(V_ext[kt][:, DH + 1:2 * DH + 1],
                                        pt[:, DH:2 * DH], s_full[kt][:])
            nc.vector.memset(V_ext[kt][:, DH:DH + 1], 1.0)
            nc.vector.memset(V_ext[kt][:, 2 * DH + 1:2 * DH + 2], 1.0)

    # ---- heads: ST -> exp (s_k folded in scale) -> mask -> O_ext -> norm ----
    O_norm = [persist.tile([128, D], F32, name=f"On{i}", tag=f"On{i}")
              for i in range(QT_TILES)]
    with tc.tile_pool(name="st_ps", bufs=3, space="PSUM") as st_pool, \
         tc.tile_pool(name="oe_ps", bufs=2, space="PSUM") as oe_pool, \
         tc.tile_pool(name="et_sb", bufs=3) as et_pool:
        for h in range(H):
            kh = h % 2
            qt_tile = QTt[h // 2]
            qsl = slice((h % 2) * 64, (h % 2) * 64 + 64)
            o_ext = [oe_pool.tile([128, 2 * DH + 2], F32, name=f"oe{mi}", tag=f"oe{mi}")
                     for mi in range(QT_TILES)]
            for kt in range(ST_TILES):
                st = st_pool.tile([128, T], F32, name="st", tag="st")
                nc.tensor.matmul(st[:],
                                 KT[kh * DH:(kh + 1) * DH, kt * 128:(kt + 1) * 128],
                                 qt_tile[qsl, :], start=True, stop=True)
                et = et_pool.tile([128, T], F32, name="et", tag="et")
                nc.scalar.activation(et[:], st[:], AF.Exp, scale=s_full[kt][:])
                nc.vector.tensor_tensor(out=et[:], in0=et[:], in1=maskT[kt][:], op=OP.mult)
                for mi in range(QT_TILES):
                    nc.tensor.matmul(o_ext[mi][:, 0:DH + 1],
                                     et[:, mi * 128:(mi + 1) * 128],
                                     V_ext[kt][:, kh * (DH + 1):(kh + 1) * (DH + 1)],
                                     start=(kt == 0), stop=(kt == ST_TILES - 1))
            for mi in range(QT_TILES):
                r = small.tile([128, 1], F32, name="orcp", tag="orcp")
                nc.vector.reciprocal(r[:], o_ext[mi][:, DH:DH + 1])
                nc.vector.tensor_scalar_mul(O_norm[mi][:, h * DH:(h + 1) * DH],
                                            o_ext[mi][:, 0:DH], r[:])

    # ---- O_norm.T -> OT [768, 256] ----
    OT = [persist.tile([128, T], F32, name=f"OT{j}", tag=f"OT{j}") for j in range(KD)]
    with tc.tile_pool(name="ot_ps", bufs=2, space="PSUM") as ot_ps:
        for j in range(KD):
            for i in range(QT_TILES):
                pt = ot_ps.tile([128, 128], F32, name="otp", tag="otp")
                nc.tensor.transpose(pt[:], O_norm[i][:, j * 128:(j + 1) * 128], ident[:])
                nc.scalar.copy(OT[j][:, i * 128:(i + 1) * 128], pt[:])

    # ---- attn @ Wo + residual -> x_att ----
    x_att = [persist.tile([128, D], F32, name=f"xa{i}", tag=f"xa{i}")
             for i in range(QT_TILES)]
    with tc.tile_pool(name="at_ps", bufs=2, space="PSUM") as at_ps:
        for mi in range(QT_TILES):
            for nj, nsl in enumerate([slice(0, 512), slice(512, 768)]):
                pa = at_ps.tile([128, nsl.stop - nsl.start], F32, name=f"pa{nj}", tag=f"pa{nj}")
                for ki in range(KD):
                    nc.tensor.matmul(pa[:], OT[ki][:, mi * 128:(mi + 1) * 128],
                                     Wo[ki][:, nsl],
                                     start=(ki == 0), stop=(ki == KD - 1))
                nc.vector.tensor_tensor(out=x_att[mi][:, nsl], in0=pa[:],
                                        in1=x_own[mi][:, nsl], op=OP.add)
            nc.sync.dma_start(io["x_att"].ap()[mi * 128:(mi + 1) * 128, :], x_att[mi][:])

    # ---- h2 (unit-rms; norm2_w folded into Rw/W1 host-side) ----
    s2 = rms_scales(x_att, QT_TILES, "s2")
    h2 = [persist.tile([128, D], F32, name=f"h2{i}", tag=f"h2{i}")
          for i in range(QT_TILES)]
    for i in range(QT_TILES):
        nc.vector.tensor_scalar_mul(h2[i][:], x_att[i][:], s2[i][:])
        nc.sync.dma_start(io["h2"].ap()[i * 128:(i + 1) * 128, :], h2[i][:])

    # ---- router logits.T [32, 256] ----
    h2T = [persist.tile([128, T], F32, name=f"h2T{j}", tag=f"h2T{j}") for j in range(KD)]
    with tc.tile_pool(name="h2t_ps", bufs=2, space="PSUM") as h2t_ps:
        for j in range(KD):
            for i in range(QT_TILES):
                pt = h2t_ps.tile([128, 128], F32, name="h2tp", tag="h2tp")
                nc.tensor.transpose(pt[:], h2[i][:, j * 128:(j + 1) * 128], ident[:])
                nc.scalar.copy(h2T[j][:, i * 128:(i + 1) * 128], pt[:])
        pl = h2t_ps.tile([32, T], F32, name="pl", tag="pl")
        for ki in range(KD):
            nc.tensor.matmul(pl[:], Rw[ki][:], h2T[ki][:],
                             start=(ki == 0), stop=(ki == KD - 1))
        lg = persist.tile([32, T], F32, name="lg", tag="lg")
        nc.scalar.copy(lg[:], pl[:])
        nc.sync.dma_start(io["logitsT"].ap()[:], lg[:])


# ---------------------------------------------------------------- kernel B --
def _build_b(C):
    nc = bacc.Bacc("TRN2", target_bir_lowering=False, debug=False,
                   num_devices=N_CORES)
    io = {}
    io["XT"] = nc.dram_tensor("XT", [EPC, D, C], BF16, kind="ExternalInput")
    io["CW"] = nc.dram_tensor("CW", [EPC, C, 1], F32, kind="ExternalInput")
    io["W1"] = nc.dram_tensor("W1", [EPC, D, F], BF16, kind="ExternalInput")
    io["W2"] = nc.dram_tensor("W2", [EPC, F, D], BF16, kind="ExternalInput")
    io["OUT"] = nc.dram_tensor("OUT", [EPC, C, D], F32, kind="ExternalOutput")
    with tile.TileContext(nc) as tc:
        with ExitStack() as ctx:
            _body_b(ctx, tc, io, C)
    nc.compile()
    return nc


def _body_b(ctx, tc, io, C):
    nc = tc.nc
    CM = C // 128
    wb = 2 if C <= 384 else 1   # extreme-skew fallback: fit SBUF, perf secondary
    xt_pool = ctx.enter_context(tc.tile_pool(name="xt", bufs=2))
    cw_pool = ctx.enter_context(tc.tile_pool(name="cw", bufs=2))
    w1_pool = ctx.enter_context(tc.tile_pool(name="w1", bufs=wb))
    w2_pool = ctx.enter_context(tc.tile_pool(name="w2", bufs=wb))
    gt_pool = ctx.enter_context(tc.tile_pool(name="gt", bufs=1))
    out_pool = ctx.enter_context(tc.tile_pool(name="out", bufs=3))
    ps1 = ctx.enter_context(tc.tile_pool(name="ps1", bufs=4, space="PSUM"))
    ps2 = ctx.enter_context(tc.tile_pool(name="ps2", bufs=2, space="PSUM"))

    for e in range(EPC):
        xt = [xt_pool.tile([128, C], BF16, name=f"xt{k}", tag=f"xt{k}") for k in range(KD)]
        for k in range(KD):
            nc.sync.dma_start(xt[k][:], io["XT"].ap()[e, k * 128:(k + 1) * 128, :])
        cw = [cw_pool.tile([128, 1], F32, name=f"cwm{m}", tag=f"cwm{m}") for m in range(CM)]
        for m in range(CM):
            nc.sync.dma_start(cw[m][:], io["CW"].ap()[e, m * 128:(m + 1) * 128, :])
        w1 = [w1_pool.tile([128, F], BF16, name=f"w1k{k}", tag=f"w1k{k}") for k in range(KD)]
        for k in range(KD):
            nc.sync.dma_start(w1[k][:], io["W1"].ap()[e, k * 128:(k + 1) * 128, :])
        w2 = [w2_pool.tile([128, D], BF16, name=f"w2k{k}", tag=f"w2k{k}") for k in range(KF)]
        for k in range(KF):
            nc.gpsimd.dma_start(w2[k][:], io["W2"].ap()[e, k * 128:(k + 1) * 128, :])

        gt = [gt_pool.tile([128, C], BF16, name=f"gt{m}", tag=f"gt{m}") for m in range(KF)]
        nchunks = [slice(i, min(i + 512, C)) for i in range(0, C, 512)]
        for mg in range(KF):
            for nsl in nchunks:
                g = ps1.tile([128, min(512, C)], F32, name="g", tag="g")
                gs = g[:, 0:nsl.stop - nsl.start]
                for ki in range(KD):
                    nc.tensor.matmul(gs, w1[ki][:, mg * 128:(mg + 1) * 128],
                                     xt[ki][:, nsl],
                                     start=(ki == 0), stop=(ki == KD - 1))
                nc.scalar.activation(gt[mg][:, nsl], gs, AF.Gelu_apprx_tanh)

        for mi in range(CM):
            for nj, nsl in enumerate([slice(0, 512), slice(512, 768)]):
                o = ps2.tile([128, nsl.stop - nsl.start], F32, name="o", tag=f"o{nj}")
                for ki in range(KF):
                    nc.tensor.matmul(o[:], gt[ki][:, mi * 128:(mi + 1) * 128],
                                     w2[ki][:, nsl],
                                     start=(ki == 0), stop=(ki == KF - 1))
                ob = out_pool.tile([128, nsl.stop - nsl.start], F32, name="ob", tag=f"ob{nj}")
                nc.vector.tensor_scalar_mul(ob[:], o[:], cw[mi][:])
                nc.sync.dma_start(
                    io["OUT"].ap()[e, mi * 128:(mi + 1) * 128, nsl], ob[:])


# -------------------------------------------------------------- host glue --
def _prep_a_maps(x, norm1_w, norm2_w, Wq, Wk, Wv, Wo, router_w):
    ck = _HOST_CACHE.get("a_w")
    if ck is not None and all(a is b for a, b in zip(
            ck["src"], (norm1_w, norm2_w, Wq, Wk, Wv, Wo, router_w))):
        Wq_p, Wk_p, Wv_p, Wo_p, Rw_p = ck["out"]
    else:
        n1 = np.asarray(norm1_w, np.float64)[:, None]
        n2 = np.asarray(norm2_w, np.float64)[:, None]
        Wq_p = (n1 * np.asarray(Wq, np.float64) * 0.125).reshape(D, H, DH)
        Wq_p = np.ascontiguousarray(Wq_p[:, HEAD_PERM, :].reshape(D, D).astype(np.float32))
        Wk_p = np.ascontiguousarray((n1 * Wk).astype(np.float32))
        Wv_p = np.ascontiguousarray((n1 * Wv).astype(np.float32))
        Wo_p = np.asarray(Wo, np.float64).reshape(H, DH, D)[HEAD_PERM].reshape(D, D)
        Wo_p = np.ascontiguousarray(Wo_p.astype(np.float32))
        Rw_p = np.ascontiguousarray((n2 * router_w).astype(np.float32))
        _HOST_CACHE["a_w"] = {
            "src": (norm1_w, norm2_w, Wq, Wk, Wv, Wo, router_w),
            "out": (Wq_p, Wk_p, Wv_p, Wo_p, Rw_p)}

    kidx = np.arange(S)[:, None]
    maps = []
    for c in range(N_CORES):
        b, blk = c // (N_CORES // B), c % (N_CORES // B)
        qidx = blk * T + np.arange(T)[None, :]
        maps.append({
            "x_own": np.ascontiguousarray(x[b, blk * T:(blk + 1) * T]),
            "x_full": np.ascontiguousarray(x[b]),
            "xT_full": np.ascontiguousarray(x[b].T),
            "maskT": (kidx <= qidx).astype(np.float32),
            "Wq": Wq_p, "Wk": Wk_p, "Wv": Wv_p, "Wo": Wo_p, "Rw": Rw_p,
        })
    return maps


def _route(logits):
    """Top-2 routing from fp32 logits (float64 softmax; matches jax.lax.top_k
    tie-breaking via stable argsort)."""
    lg = logits.astype(np.float64)
    p = np.exp(lg - lg.max(-1, keepdims=True))
    p /= p.sum(-1, keepdims=True)
    order = np.argsort(-p, axis=-1, kind="stable")
    top2 = order[:, :TOP_K]
    vals = np.take_along_axis(p, top2, axis=-1)
    vals = vals / vals.sum(-1, keepdims=True)
    idx_e = [np.where((top2 == e).any(-1))[0] for e in range(E)]
    w_e = []
    for e in range(E):
        m = top2[idx_e[e]] == e
        w_e.append(np.where(m[:, 0], vals[idx_e[e], 0], vals[idx_e[e], 1]))
    counts = np.array([len(i) for i in idx_e])
    # aux loss: E * sum(frac_routed * mean_prob)
    aux = np.float32(E * np.sum((counts / lg.shape[0]) * p.mean(0)))
    return idx_e, w_e, counts, aux


def _run(nc, maps, label):
    global LAST_EXEC_NS
    res = run_bass_kernel_spmd(nc, maps, core_ids=list(range(N_CORES)))
    if res.exec_time_ns is not None:
        LAST_EXEC_NS[label] = res.exec_time_ns
    return res.results


def kernel(x, norm1_w, norm2_w, Wq, Wk, Wv, Wo, router_w, W1, W2):
    x = np.asarray(x, np.float32)
    if "A" not in _CACHE:
        _CACHE["A"] = _build_a()
    maps_a = _prep_a_maps(x, norm1_w, norm2_w, Wq, Wk, Wv, Wo, router_w)
    res_a = _run(_CACHE["A"], maps_a, "A")

    x_att = np.empty((B * S, D), np.float32)
    h2 = np.empty((B * S, D), np.float32)
    logits = np.empty((B * S, E), np.float32)
    for c in range(N_CORES):
        sl = slice(c * T, (c + 1) * T)
        x_att[sl] = res_a[c]["x_att"]
        h2[sl] = res_a[c]["h2"]
        logits[sl] = np.asarray(res_a[c]["logitsT"]).T

    idx_e, w_e, counts, aux = _route(logits)
    C = max(256, int(np.ceil(counts.max() / 128)) * 128)

    ck = _HOST_CACHE.get("b_w")
    if ck is not None and ck["src"][0] is W1 and ck["src"][1] is W2 \
            and ck["src"][2] is norm2_w:
        W1b, W2b = ck["out"]
    else:
        n2 = np.asarray(norm2_w, np.float64)[:, None]
        W1b = np.ascontiguousarray((n2 * np.asarray(W1, np.float64)).astype(ml_dtypes.bfloat16))
        W2b = np.ascontiguousarray(np.asarray(W2).astype(ml_dtypes.bfloat16))
        _HOST_CACHE["b_w"] = {"src": (W1, W2, norm2_w), "out": (W1b, W2b)}

    XT = np.zeros((E, D, C), ml_dtypes.bfloat16)
    CW = np.zeros((E, C, 1), np.float32)
    for e in range(E):
        n = counts[e]
        XT[e, :, :n] = h2[idx_e[e]].T.astype(ml_dtypes.bfloat16)
        CW[e, :n, 0] = w_e[e]

    key_b = ("B", C)
    if key_b not in _CACHE:
        _CACHE[key_b] = _build_b(C)
    maps_b = []
    for c in range(N_CORES):
        es = slice(c * EPC, (c + 1) * EPC)
        maps_b.append({
            "XT": np.ascontiguousarray(XT[es]),
            "CW": np.ascontiguousarray(CW[es]),
            "W1": np.ascontiguousarray(W1b[es]),
            "W2": np.ascontiguousarray(W2b[es]),
        })
    res_b = _run(_CACHE[key_b], maps_b, "B")

    y = np.zeros((B * S, D), np.float32)
    for e in range(E):
        c, j = e // EPC, e % EPC
        n = counts[e]
        if n:
            y[idx_e[e]] += np.asarray(res_b[c]["OUT"])[j, :n]

    out = (x_att + y).reshape(B, S, D).astype(np.float32)
    return out, aux
